# revision 6
# baseline (speedup 1.0000x reference)
"""DeepSpeedAttention (B=2, S=2048, H=4096, 32 heads) on 8 Trainium2 cores.

Sharding: tensor-parallel across heads. Each core computes QKV for its 4
heads (column shard of attn_qkvw), full attention for those heads, and a
partial output projection (row shard of attn_ow). The 8 partial outputs are
summed on the host (host-side all-reduce) and the output bias is added.

Device kernel layout choices (per core):
  xT   [4096 H, 4096 tok]   bf16  (x transposed host-side; replicated)
  wq/wk[4096 H, 512]        bf16  (Q/K column shards)
  wv   [4096 H, 512]        bf16
  bq/bk/bv [1, 512]         bf16
  wo   [512, 4096]          bf16  (row shard of attn_ow)
  out  [4096 tok, 4096]     f32   (partial result, summed on host)

Phase A: QKV projection.
  qT,kT computed directly transposed ([col, tok]) by making the weight the
  stationary operand; v computed natural ([tok, col]). Biases are added via
  rank-1 matmuls accumulated into the same PSUM group. Results staged to DRAM.
Phase B: attention per (batch, local head).
  scoresT[k,q] = kT_tile^T @ qT (d contraction on partitions), exp on ACT
  (scale folded in), PV accumulated with v as stationary -> ctxT[d,q] in PSUM.
  Softmax denominator: strided DVE reduce over the 16 k-tiles, then a
  [128->1] ones-matmul for the cross-partition sum, reciprocal_approx_fast,
  and a rank-1 matmul to broadcast 1/l across partitions; ctxT is scaled
  during PSUM eviction.
Phase C: output projection from resident ctxT, partial f32 result to DRAM.
"""

import os
import numpy as np
import ml_dtypes
from contextlib import ExitStack

try:
    import jax
    jax.config.update(
        "jax_compilation_cache_dir", os.path.expanduser("~/.bass_jax_cache"))
    jax.config.update("jax_persistent_cache_min_compile_time_secs", 10.0)
    jax.config.update("jax_persistent_cache_min_entry_size_bytes", 0)
except Exception:
    pass

import concourse.bass as bass
import concourse.tile as tile
from concourse import bacc, mybir
from concourse.bass_utils import run_bass_kernel_spmd

BF16 = mybir.dt.bfloat16
F32 = mybir.dt.float32
AF = mybir.ActivationFunctionType

H = 4096          # hidden
TOK = 4096        # B*S tokens
S = 2048          # seq len per batch
NB = 2            # batches
HL = 4            # heads per core
HD = 128          # head dim
COLS = HL * HD    # per-core hidden shard (512)
NCORES = 8
KT = H // 128     # 32 contraction tiles for the projections
SCALE = 1.0 / float(np.sqrt(HD))


def build_nc(phases: str = "ABC"):
    nc = bacc.Bacc("TRN2", target_bir_lowering=False, debug=False)

    xT = nc.dram_tensor("xT", [H, TOK], BF16, kind="ExternalInput").ap()
    wq = nc.dram_tensor("wq", [H, COLS], BF16, kind="ExternalInput").ap()
    wk = nc.dram_tensor("wk", [H, COLS], BF16, kind="ExternalInput").ap()
    wv = nc.dram_tensor("wv", [H, COLS], BF16, kind="ExternalInput").ap()
    bq = nc.dram_tensor("bq", [1, COLS], BF16, kind="ExternalInput").ap()
    bk = nc.dram_tensor("bk", [1, COLS], BF16, kind="ExternalInput").ap()
    bv = nc.dram_tensor("bv", [1, COLS], BF16, kind="ExternalInput").ap()
    wo = nc.dram_tensor("wo", [COLS, H], BF16, kind="ExternalInput").ap()
    out = nc.dram_tensor("out", [TOK, H], F32, kind="ExternalOutput").ap()

    with tile.TileContext(nc) as tc, ExitStack() as ctx:
        dram = ctx.enter_context(tc.tile_pool(name="dram", bufs=1, space="DRAM"))
        qT_d = dram.tile([COLS, TOK], BF16)
        kT_d = dram.tile([COLS, TOK], BF16)
        v_d = dram.tile([TOK, COLS], BF16)

        const = ctx.enter_context(tc.tile_pool(name="const", bufs=1))
        ones_bf = const.tile([1, 512], BF16)
        nc.vector.memset(ones_bf[:], 1.0)
        ones_col = const.tile([128, 1], F32)
        nc.vector.memset(ones_col[:], 1.0)
        ones_row = const.tile([1, 128], F32)
        nc.vector.memset(ones_row[:], 1.0)
        bq_sb = const.tile([1, COLS], BF16)
        nc.sync.dma_start(bq_sb[:], bq)
        bk_sb = const.tile([1, COLS], BF16)
        nc.sync.dma_start(bk_sb[:], bk)
        bv_sb = const.tile([1, COLS], BF16)
        nc.sync.dma_start(bv_sb[:], bv)

        # ---------------- Phase A: QKV projection ----------------
        if "A" in phases:
         with tc.tile_pool(name="aw", bufs=1) as awp, \
             tc.tile_pool(name="ax", bufs=2) as axp, \
             tc.tile_pool(name="ast", bufs=6) as astp, \
             tc.tile_pool(name="aps", bufs=3, space="PSUM") as apsp:
            wq_sb = awp.tile([128, KT, COLS], BF16)
            nc.sync.dma_start(wq_sb[:], wq.rearrange("(kt p) c -> p kt c", p=128))
            wk_sb = awp.tile([128, KT, COLS], BF16)
            nc.sync.dma_start(wk_sb[:], wk.rearrange("(kt p) c -> p kt c", p=128))
            wv_sb = awp.tile([128, KT, COLS], BF16)
            nc.sync.dma_start(wv_sb[:], wv.rearrange("(kt p) c -> p kt c", p=128))

            for tck in range(TOK // 512):
                t0 = tck * 512
                x_sb = axp.tile([128, KT, 512], BF16)
                nc.sync.dma_start(
                    x_sb[:],
                    xT[:, t0:t0 + 512].rearrange("(kt p) t -> p kt t", p=128),
                )
                # qT / kT: [col-tile 128, tok 512], weight stationary
                for ct in range(8):
                    is_q = ct < 4
                    w_sb = wq_sb if is_q else wk_sb
                    b_sb = bq_sb if is_q else bk_sb
                    dst = qT_d if is_q else kT_d
                    c0 = (ct % 4) * 128
                    ps = apsp.tile([128, 512], F32, tag="qk")
                    for kt in range(KT):
                        nc.tensor.matmul(
                            ps[:], w_sb[:, kt, c0:c0 + 128], x_sb[:, kt, :],
                            start=(kt == 0), stop=False,
                        )
                    # bias: out[col, tok] += b[col] x ones[tok]
                    nc.tensor.matmul(
                        ps[:], b_sb[:, c0:c0 + 128], ones_bf[:],
                        start=False, stop=True,
                    )
                    st = astp.tile([128, 512], BF16, tag="qk_st")
                    nc.scalar.copy(st[:], ps[:])
                    nc.sync.dma_start(dst[c0:c0 + 128, t0:t0 + 512], st[:])
                # v: [tok-tile 128, col 512], x stationary
                for tt in range(4):
                    ps = apsp.tile([128, 512], F32, tag="v")
                    for kt in range(KT):
                        nc.tensor.matmul(
                            ps[:], x_sb[:, kt, tt * 128:(tt + 1) * 128],
                            wv_sb[:, kt, :],
                            start=(kt == 0), stop=False,
                        )
                    # bias: out[tok, col] += ones[tok] x b[col]
                    nc.tensor.matmul(
                        ps[:], ones_bf[:, 0:128], bv_sb[:],
                        start=False, stop=True,
                    )
                    st = astp.tile([128, 512], BF16, tag="v_st")
                    nc.scalar.copy(st[:], ps[:])
                    nc.sync.dma_start(
                        v_d[t0 + tt * 128:t0 + (tt + 1) * 128, :], st[:])

        # ---------------- Phase B: attention ----------------
        # ctxT survives phase B into phase C: [d, head, tok]. Opened after
        # phase A's pools release so its 32KB/partition reuses their space.
        ctxp = ctx.enter_context(tc.tile_pool(name="ctxp", bufs=1))
        ctxT = ctxp.tile([128, HL, TOK], BF16)

        NKT = S // 128  # 16 k tiles per batch
        if "B" in phases:
         with tc.tile_pool(name="bqk", bufs=2) as bqkp, \
             tc.tile_pool(name="bpr", bufs=2) as bprp, \
             tc.tile_pool(name="bst", bufs=3) as bstp, \
             tc.tile_pool(name="bsc", bufs=3, space="PSUM") as bscp, \
             tc.tile_pool(name="bcx", bufs=2, space="PSUM") as bcxp, \
             tc.tile_pool(name="bls", bufs=1, space="PSUM") as blsp, \
             tc.tile_pool(name="brp", bufs=1, space="PSUM") as brpp:
            for b in range(NB):
                for hl in range(HL):
                    r0 = hl * 128
                    s0 = b * S
                    qh = bqkp.tile([128, S], BF16, tag="qh")
                    nc.sync.dma_start(qh[:], qT_d[r0:r0 + 128, s0:s0 + S])
                    kh = bqkp.tile([128, S], BF16, tag="kh")
                    nc.sync.dma_start(kh[:], kT_d[r0:r0 + 128, s0:s0 + S])
                    vh = bqkp.tile([128, NKT, 128], BF16, tag="vh")
                    nc.sync.dma_start(
                        vh[:],
                        v_d[s0:s0 + S, r0:r0 + 128].rearrange(
                            "(i p) d -> p i d", p=128),
                    )
                    for qc in range(S // 512):
                        q0 = qc * 512
                        probs = bprp.tile([128, NKT, 512], BF16, tag="probs")
                        cps = bcxp.tile([128, 512], F32, tag="ctx")
                        for ki in range(NKT):
                            sps = bscp.tile([128, 512], F32, tag="sc")
                            nc.tensor.matmul(
                                sps[:], kh[:, ki * 128:(ki + 1) * 128],
                                qh[:, q0:q0 + 512], start=True, stop=True,
                            )
                            nc.scalar.activation(
                                probs[:, ki, :], sps[:], AF.Exp, scale=SCALE)
                            nc.tensor.matmul(
                                cps[:], vh[:, ki, :], probs[:, ki, :],
                                start=(ki == 0), stop=(ki == NKT - 1),
                            )
                        # denominator: sum over k = sum over 16 tiles (DVE,
                        # strided innermost) then over 128 partitions (PE)
                        lacc = bstp.tile([128, 512], F32, tag="lacc")
                        nc.vector.tensor_reduce(
                            lacc[:], probs[:].rearrange("p k q -> p q k"),
                            axis=mybir.AxisListType.X, op=mybir.AluOpType.add,
                        )
                        lsum = blsp.tile([1, 512], F32, tag="ls")
                        nc.tensor.matmul(
                            lsum[:], ones_col[:], lacc[:], start=True, stop=True)
                        rec = bstp.tile([1, 512], F32, tag="rec")
                        nc.vector.reciprocal_approx_fast(out=rec[:], in_=lsum[:])
                        rps = brpp.tile([128, 512], F32, tag="rp")
                        nc.tensor.matmul(
                            rps[:], ones_row[:], rec[:], start=True, stop=True)
                        rsb = bstp.tile([128, 512], F32, tag="rsb")
                        nc.scalar.copy(rsb[:], rps[:])
                        nc.vector.tensor_mul(
                            ctxT[:, hl, s0 + q0:s0 + q0 + 512], cps[:], rsb[:])

        # ---------------- Phase C: output projection ----------------
        if "C" in phases:
         with tc.tile_pool(name="cw", bufs=1) as cwp, \
             tc.tile_pool(name="cst", bufs=6) as cstp, \
             tc.tile_pool(name="cps", bufs=6, space="PSUM") as cpsp:
            wo_sb = cwp.tile([128, HL, H], BF16)
            nc.sync.dma_start(wo_sb[:], wo.rearrange("(hl p) n -> p hl n", p=128))
            for ot in range(TOK // 128):
                t0 = ot * 128
                for ncol in range(H // 512):
                    n0 = ncol * 512
                    ps = cpsp.tile([128, 512], F32, tag="op")
                    for hl in range(HL):
                        nc.tensor.matmul(
                            ps[:], ctxT[:, hl, t0:t0 + 128],
                            wo_sb[:, hl, n0:n0 + 512],
                            start=(hl == 0), stop=(hl == HL - 1),
                        )
                    st = cstp.tile([128, 512], F32, tag="ost")
                    if (ot + ncol) % 2 == 0:
                        nc.scalar.copy(st[:], ps[:])
                    else:
                        nc.vector.tensor_copy(st[:], ps[:])
                    nc.sync.dma_start(out[t0:t0 + 128, n0:n0 + 512], st[:])

    nc.compile()
    return nc


_NC = None


def _get_nc():
    global _NC
    if _NC is None:
        _NC = build_nc()
    return _NC


def _shard_inputs(x, attn_qkvw, attn_qkvb, attn_ow):
    bf = ml_dtypes.bfloat16
    x = np.asarray(x, dtype=np.float32)
    w = np.asarray(attn_qkvw, dtype=np.float32)
    b = np.asarray(attn_qkvb, dtype=np.float32)
    wo = np.asarray(attn_ow, dtype=np.float32)

    xT = np.ascontiguousarray(x.reshape(TOK, H).T).astype(bf)
    w4 = w.reshape(H, 3, 32, HD)
    b4 = b.reshape(3, 32, HD)
    in_maps = []
    for c in range(NCORES):
        hs = slice(c * HL, (c + 1) * HL)
        in_maps.append({
            "xT": xT,
            "wq": np.ascontiguousarray(w4[:, 0, hs, :].reshape(H, COLS)).astype(bf),
            "wk": np.ascontiguousarray(w4[:, 1, hs, :].reshape(H, COLS)).astype(bf),
            "wv": np.ascontiguousarray(w4[:, 2, hs, :].reshape(H, COLS)).astype(bf),
            "bq": b4[0, hs, :].reshape(1, COLS).astype(bf),
            "bk": b4[1, hs, :].reshape(1, COLS).astype(bf),
            "bv": b4[2, hs, :].reshape(1, COLS).astype(bf),
            "wo": np.ascontiguousarray(
                wo[c * COLS:(c + 1) * COLS, :]).astype(bf),
        })
    return in_maps


def kernel(x, attn_qkvw, attn_qkvb, attn_ow, attn_ob):
    nc = _get_nc()
    in_maps = _shard_inputs(x, attn_qkvw, attn_qkvb, attn_ow)
    res = run_bass_kernel_spmd(nc, in_maps, core_ids=list(range(NCORES)))
    acc = res.results[0]["out"]
    for c in range(1, NCORES):
        acc = acc + res.results[c]["out"]
    acc = acc + np.asarray(attn_ob, dtype=np.float32)[None, :]
    return acc.reshape(NB, S, H)


# revision 22
# speedup vs baseline: 15400.8256x; 15400.8256x over previous
"""DeepSpeedAttention (B=2, S=2048, H=4096, 32 heads) on 8 Trainium2 cores.

Sharding: tensor-parallel across heads. Each core computes QKV for its 4
heads (column shard of attn_qkvw), full attention for those heads, and a
partial output projection (row shard of attn_ow). The 8 partial outputs are
summed on the host (host-side all-reduce) and the output bias is added.

Device kernel layout choices (per core):
  xT   [4096 H, 4096 tok]   bf16  (x transposed host-side; replicated)
  wq/wk[4096 H, 512]        bf16  (Q/K column shards)
  wv   [4096 H, 512]        bf16
  bq/bk/bv [1, 512]         bf16
  wo   [512, 4096]          bf16  (row shard of attn_ow)
  out  [4096 tok, 4096]     f32   (partial result, summed on host)

Phase A: QKV projection.
  qT,kT computed directly transposed ([col, tok]) by making the weight the
  stationary operand; v computed natural ([tok, col]). Biases are added via
  rank-1 matmuls accumulated into the same PSUM group. Results staged to DRAM.
Phase B: attention per (batch, local head).
  scoresT[k,q] = kT_tile^T @ qT (d contraction on partitions); exp on ACT
  1024-wide over two-score-tile PSUM pairs (softmax scale folded into the
  activation); PV accumulated with v stationary -> ctxT[d,q] in PSUM.
  Softmax denominator (no max subtraction needed -- scores are ~N(0,1)):
  bf16 pairwise adds + f32 tree on DVE, cross-partition sum broadcast via
  gpsimd.partition_all_reduce, reciprocal_approx_fast; ctxT is scaled by
  1/l during PSUM eviction.
Phase C: output projection from resident ctxT, partial f32 result to DRAM.
  Phase C shares the pool scope with phase B so its matmuls fill phase B's
  TensorE gaps once batch-0 ctxT regions complete.
"""

import os
import numpy as np
import ml_dtypes
from contextlib import ExitStack

try:
    import jax
    jax.config.update(
        "jax_compilation_cache_dir", os.path.expanduser("~/.bass_jax_cache"))
    jax.config.update("jax_persistent_cache_min_compile_time_secs", 10.0)
    jax.config.update("jax_persistent_cache_min_entry_size_bytes", 0)
except Exception:
    pass

import concourse.bass as bass
from concourse import bass_isa
import concourse.tile as tile
from concourse import bacc, mybir
from concourse.bass_utils import run_bass_kernel_spmd

BF16 = mybir.dt.bfloat16
F32 = mybir.dt.float32
AF = mybir.ActivationFunctionType

H = 4096          # hidden
TOK = 4096        # B*S tokens
S = 2048          # seq len per batch
NB = 2            # batches
HL = 4            # heads per core
HD = 128          # head dim
COLS = HL * HD    # per-core hidden shard (512)
NCORES = 8
KT = H // 128     # 32 contraction tiles for the projections
SCALE = 1.0 / float(np.sqrt(HD))


def build_nc(phases: str = "ABC"):
    nc = bacc.Bacc("TRN2", target_bir_lowering=False, debug=False)

    xT = nc.dram_tensor("xT", [H, TOK], BF16, kind="ExternalInput").ap()
    wq = nc.dram_tensor("wq", [H, COLS], BF16, kind="ExternalInput").ap()
    wk = nc.dram_tensor("wk", [H, COLS], BF16, kind="ExternalInput").ap()
    wv = nc.dram_tensor("wv", [H, COLS], BF16, kind="ExternalInput").ap()
    bq = nc.dram_tensor("bq", [1, COLS], F32, kind="ExternalInput").ap()
    bk = nc.dram_tensor("bk", [1, COLS], F32, kind="ExternalInput").ap()
    bv = nc.dram_tensor("bv", [1, COLS], BF16, kind="ExternalInput").ap()
    wo = nc.dram_tensor("wo", [COLS, H], BF16, kind="ExternalInput").ap()
    out = nc.dram_tensor("out", [TOK, H], F32, kind="ExternalOutput").ap()

    with tile.TileContext(nc) as tc, ExitStack() as ctx:
        dram = ctx.enter_context(tc.tile_pool(name="dram", bufs=1, space="DRAM"))
        qT_d = dram.tile([COLS, TOK], BF16)
        kT_d = dram.tile([COLS, TOK], BF16)
        v_d = dram.tile([TOK, COLS], BF16)

        const = ctx.enter_context(tc.tile_pool(name="const", bufs=1))
        ones_bf = const.tile([1, 512], BF16)
        nc.vector.memset(ones_bf[:], 1.0)
        # per-partition layout [col-within-tile, col-tile] for tensor_scalar
        bq_sb = const.tile([128, HL], F32)
        nc.sync.dma_start(bq_sb[:], bq.rearrange("o (ct p) -> p (o ct)", p=128))
        bk_sb = const.tile([128, HL], F32)
        nc.sync.dma_start(bk_sb[:], bk.rearrange("o (ct p) -> p (o ct)", p=128))
        bv_sb = const.tile([1, COLS], BF16)
        nc.sync.dma_start(bv_sb[:], bv)

        # Warm-start pool for phase B's first (batch, head): allocated
        # before phase A's pools so its addresses don't overlap them and its
        # DMAs can start mid-phase-A (no SBUF-release wait at the A->B seam).
        bqk0 = ctx.enter_context(tc.tile_pool(name="bqk0", bufs=1))

        # ---------------- Phase A: QKV projection ----------------
        if "A" in phases:
         with tc.tile_pool(name="aw", bufs=1) as awp, \
             tc.tile_pool(name="ax", bufs=2) as axp, \
             tc.tile_pool(name="ast", bufs=6) as astp, \
             tc.tile_pool(name="aps", bufs=3, space="PSUM") as apsp:
            # wq + first x chunk first so the q matmuls can start ASAP
            wq_sb = awp.tile([128, KT, COLS], BF16)
            nc.sync.dma_start(wq_sb[:], wq.rearrange("(kt p) c -> p kt c", p=128))
            x0_sb = axp.tile([128, KT, 512], BF16, tag="x")
            nc.sync.dma_start(
                x0_sb[:], xT[:, 0:512].rearrange("(kt p) t -> p kt t", p=128))
            wk_sb = awp.tile([128, KT, COLS], BF16)
            nc.sync.dma_start(wk_sb[:], wk.rearrange("(kt p) c -> p kt c", p=128))
            wv_sb = awp.tile([128, KT, COLS], BF16)
            nc.sync.dma_start(wv_sb[:], wv.rearrange("(kt p) c -> p kt c", p=128))

            for tck in range(TOK // 512):
                t0 = tck * 512
                if tck == 0:
                    x_sb = x0_sb
                else:
                    x_sb = axp.tile([128, KT, 512], BF16, tag="x")
                    nc.sync.dma_start(
                        x_sb[:],
                        xT[:, t0:t0 + 512].rearrange("(kt p) t -> p kt t", p=128),
                    )
                # qT / kT: [col-tile 128, tok 512], weight stationary
                for ct in range(8):
                    is_q = ct < 4
                    w_sb = wq_sb if is_q else wk_sb
                    b_sb = bq_sb if is_q else bk_sb
                    dst = qT_d if is_q else kT_d
                    c0 = (ct % 4) * 128
                    ps = apsp.tile([128, 512], F32, tag="qk")
                    for kt in range(KT):
                        nc.tensor.matmul(
                            ps[:], w_sb[:, kt, c0:c0 + 128], x_sb[:, kt, :],
                            start=(kt == 0), stop=(kt == KT - 1),
                        )
                    # eviction on DVE (idle in phase A) with fused bias add
                    st = astp.tile([128, 512], BF16, tag="qk_st")
                    nc.vector.tensor_scalar_add(
                        st[:], ps[:], b_sb[:, ct % 4:ct % 4 + 1])
                    nc.sync.dma_start(dst[c0:c0 + 128, t0:t0 + 512], st[:])
                # v: [tok-tile 128, col 512], x stationary
                for tt in range(4):
                    ps = apsp.tile([128, 512], F32, tag="v")
                    for kt in range(KT):
                        nc.tensor.matmul(
                            ps[:], x_sb[:, kt, tt * 128:(tt + 1) * 128],
                            wv_sb[:, kt, :],
                            start=(kt == 0), stop=False,
                        )
                    # bias: out[tok, col] += ones[tok] x b[col]
                    nc.tensor.matmul(
                        ps[:], ones_bf[:, 0:128], bv_sb[:],
                        start=False, stop=True,
                    )
                    st = astp.tile([128, 512], BF16, tag="v_st")
                    nc.vector.tensor_copy(st[:], ps[:])
                    nc.sync.dma_start(
                        v_d[t0 + tt * 128:t0 + (tt + 1) * 128, :], st[:])

        # ---------------- Phase B: attention ----------------
        # ctxT survives phase B into phase C: [d, head, tok]. Opened after
        # phase A's pools release so its 32KB/partition reuses their space.
        ctxp = ctx.enter_context(tc.tile_pool(name="ctxp", bufs=1))
        ctxT = ctxp.tile([128, HL, TOK], BF16)

        NKT = S // 128  # 16 k tiles per batch
        # wo is loaded ahead of phase B so phase C can start immediately
        cwp = ctx.enter_context(tc.tile_pool(name="cw", bufs=1))
        wo_sb = cwp.tile([128, HL, H], BF16)
        nc.sync.dma_start(wo_sb[:], wo.rearrange("(hl p) n -> p hl n", p=128))

        with tc.tile_pool(name="bqk", bufs=2) as bqkp, \
             tc.tile_pool(name="bpr", bufs=3) as bprp, \
             tc.tile_pool(name="bst", bufs=1) as bstp, \
             tc.tile_pool(name="bsc", bufs=2, space="PSUM") as bscp, \
             tc.tile_pool(name="bcx", bufs=2, space="PSUM") as bcxp, \
             tc.tile_pool(name="cst", bufs=4) as cstp, \
             tc.tile_pool(name="cps", bufs=2, space="PSUM") as cpsp:
          if "B" in phases:
            for b in range(NB):
                for hl in range(HL):
                    r0 = hl * 128
                    s0 = b * S
                    pool_i = bqk0 if (b == 0 and hl == 0) else bqkp
                    qh = pool_i.tile([128, S], BF16, tag="qh")
                    nc.sync.dma_start(qh[:], qT_d[r0:r0 + 128, s0:s0 + S])
                    kh = pool_i.tile([128, S], BF16, tag="kh")
                    nc.sync.dma_start(kh[:], kT_d[r0:r0 + 128, s0:s0 + S])
                    vh = pool_i.tile([128, NKT, 128], BF16, tag="vh")
                    nc.sync.dma_start(
                        vh[:],
                        v_d[s0:s0 + S, r0:r0 + 128].rearrange(
                            "(i p) d -> p i d", p=128),
                    )
                    for qc in range(S // 512):
                        q0 = qc * 512
                        probs = bprp.tile([128, NKT, 512], BF16, tag="probs")
                        tmp8 = bprp.tile([128, NKT // 2, 512], BF16, tag="tmp8", bufs=2)
                        cps = bcxp.tile([128, 512], F32, tag="ctx")
                        for kj in range(NKT // 2):
                            # two score tiles in one 2-bank PSUM tile so the
                            # exp runs 1024 wide
                            sps = bscp.tile([128, 2, 512], F32, tag="sc")
                            for u in range(2):
                                ki = 2 * kj + u
                                nc.tensor.matmul(
                                    sps[:, u, :],
                                    kh[:, ki * 128:(ki + 1) * 128],
                                    qh[:, q0:q0 + 512], start=True, stop=True,
                                )
                            nc.scalar.activation(
                                probs[:, 2 * kj:2 * kj + 2, :], sps[:],
                                AF.Exp, scale=SCALE)
                            for u in range(2):
                                ki = 2 * kj + u
                                nc.tensor.matmul(
                                    cps[:], vh[:, ki, :], probs[:, ki, :],
                                    start=(ki == 0), stop=(ki == NKT - 1),
                                )
                            # denominator: pairwise sums on DVE (f32)
                            nc.vector.tensor_add(
                                tmp8[:, kj, :], probs[:, 2 * kj, :],
                                probs[:, 2 * kj + 1, :])
                        # reduction tree: 8 -> 4 -> 2 -> 1 (strided DVE ops)
                        t4 = bstp.tile([128, 4, 512], F32, tag="t4")
                        nc.vector.tensor_add(
                            t4[:], tmp8[:, 0:8:2, :], tmp8[:, 1:8:2, :])
                        t2 = bstp.tile([128, 2, 512], F32, tag="t2")
                        nc.vector.tensor_add(
                            t2[:], t4[:, 0:4:2, :], t4[:, 1:4:2, :])
                        acc = bstp.tile([128, 512], F32, tag="acc")
                        nc.vector.tensor_add(acc[:], t2[:, 0, :], t2[:, 1, :])
                        # cross-partition sum (broadcast result) on GpSimd
                        lsb = bstp.tile([128, 512], F32, tag="lsb")
                        nc.gpsimd.partition_all_reduce(
                            lsb[:], acc[:], channels=128,
                            reduce_op=bass_isa.ReduceOp.add)
                        rec = bstp.tile([128, 512], F32, tag="rec")
                        nc.vector.reciprocal_approx_fast(out=rec[:], in_=lsb[:])
                        nc.vector.tensor_mul(
                            ctxT[:, hl, s0 + q0:s0 + q0 + 512], cps[:], rec[:])

          # ---------------- Phase C: output projection ----------------
          if "C" in phases:
            for ot in range(TOK // 128):
                t0 = ot * 128
                for ncol in range(H // 512):
                    n0 = ncol * 512
                    ps = cpsp.tile([128, 512], F32, tag="op")
                    for hl in range(HL):
                        nc.tensor.matmul(
                            ps[:], ctxT[:, hl, t0:t0 + 128],
                            wo_sb[:, hl, n0:n0 + 512],
                            start=(hl == 0), stop=(hl == HL - 1),
                        )
                    st = cstp.tile([128, 512], F32, tag="ost")
                    if (ot + ncol) % 2 == 0:
                        nc.scalar.copy(st[:], ps[:])
                    else:
                        nc.vector.tensor_copy(st[:], ps[:])
                    nc.sync.dma_start(out[t0:t0 + 128, n0:n0 + 512], st[:])

    nc.compile()
    return nc


_NC = None


def _get_nc():
    global _NC
    if _NC is None:
        _NC = build_nc()
    return _NC


def _shard_inputs(x, attn_qkvw, attn_qkvb, attn_ow):
    bf = ml_dtypes.bfloat16
    x = np.asarray(x, dtype=np.float32)
    w = np.asarray(attn_qkvw, dtype=np.float32)
    b = np.asarray(attn_qkvb, dtype=np.float32)
    wo = np.asarray(attn_ow, dtype=np.float32)

    xT = np.ascontiguousarray(x.reshape(TOK, H).T).astype(bf)
    w4 = w.reshape(H, 3, 32, HD)
    b4 = b.reshape(3, 32, HD)
    in_maps = []
    for c in range(NCORES):
        hs = slice(c * HL, (c + 1) * HL)
        in_maps.append({
            "xT": xT,
            "wq": np.ascontiguousarray(w4[:, 0, hs, :].reshape(H, COLS)).astype(bf),
            "wk": np.ascontiguousarray(w4[:, 1, hs, :].reshape(H, COLS)).astype(bf),
            "wv": np.ascontiguousarray(w4[:, 2, hs, :].reshape(H, COLS)).astype(bf),
            "bq": np.ascontiguousarray(b4[0, hs, :].reshape(1, COLS)),
            "bk": np.ascontiguousarray(b4[1, hs, :].reshape(1, COLS)),
            "bv": b4[2, hs, :].reshape(1, COLS).astype(bf),
            "wo": np.ascontiguousarray(
                wo[c * COLS:(c + 1) * COLS, :]).astype(bf),
        })
    return in_maps


def kernel(x, attn_qkvw, attn_qkvb, attn_ow, attn_ob):
    nc = _get_nc()
    in_maps = _shard_inputs(x, attn_qkvw, attn_qkvb, attn_ow)
    res = run_bass_kernel_spmd(nc, in_maps, core_ids=list(range(NCORES)))
    acc = res.results[0]["out"]
    for c in range(1, NCORES):
        acc = acc + res.results[c]["out"]
    acc = acc + np.asarray(attn_ob, dtype=np.float32)[None, :]
    return acc.reshape(NB, S, H)
